# revision 124
# baseline (speedup 1.0000x reference)
"""Multi-head attention (B=4, S=2048, D=1024, 16 heads) on 8 TRN2 NeuronCores.

Sharding: data-parallel over batch (4) x tensor-parallel over heads (2 groups
of 8).  Core c handles batch c//2, head-group c%2.  Each core computes its
head-group's attention and the partial output projection through its slice of
Wo; the host sums the two partials per batch and adds bo.

Per-core kernel (all matmuls bf16, fp32 accumulation):
  - x^T built on-device via PE transposes (inputs cast fp32->bf16 by the
    SWDGE DMA on the otherwise-idle GPSIMD engine; two s-row blocks per
    cast DMA to halve the ~1us SWDGE issue cost that paces the loads)
  - Q^T,K^T = (x Wq/Wk)^T via transposed projections; V natural, augmented
    with a ones column per head (yields softmax denominators for free)
  - scores^T = K_h Q_h^T per head, two heads row-packed in the 128-wide PE
  - softmax without max subtraction (scores ~ N(0,1), no overflow): exp on
    ACT with the 1/sqrt(dk) scale folded in, mask applied as a bf16
    multiply-by-complement on DVE; the complement (mask==0) is produced per
    sq-block just in time, bounced through DRAM, and read back with the DMA
    xbar transpose so mask DMA streams during attention.  The j=0 chain
    issues behind the wv loads so its serialized DMA round trips overlap
    V's projection instead of gating the first exp
  - out_h^T = V_aug^T P^T accumulated on PE; row 64 = denominator.  PV
    matmuls for step bt are emitted AFTER the scores matmuls for step bt+1
    (software pipelining) so the next exp's PSUM input is ready the moment
    ACT finishes the previous exp
  - normalize: DVE reciprocal + rank-1 PE broadcast matmul + DVE
    scalar_tensor_tensor (fused PSUM read, scale, bf16 store)
  - partial out = X Wo_slice in natural layout, DMA'd out in fp32; each
    sq-block's output projection is split into per-(st,nb) groups and
    interleaved one group per 4th bt slot of the NEXT sq block, so the
    j-boundary never clusters 32 matmuls in front of the next exps
  - K projection: only pair 0 / key-block 0 is built ahead of attention;
    pair 0's remaining key blocks stream just-ahead of their first use
    inside pr0's bt loop, and pairs 1-3 are emitted at their pr's turn in
    j0.  The attention-phase psum->SBUF results go through DVE
    (tensor_scalar with per-partition bias) because ACT is exp-saturated
    there and GPSIMD may not read PSUM (BIR verifier rejects it; the
    TimelineSim cost model does not know that constraint — always compile
    before trusting a sim win)

The walrus build here accepts at most one semaphore wait per instruction;
split_excess_waits() hoists extra waits onto NoOp carriers post-scheduling.
"""

import sys

if "/opt/trn_rl_repo" not in sys.path:
    sys.path.insert(0, "/opt/trn_rl_repo")

from contextlib import ExitStack

import numpy as np

import concourse.bass as bass
import concourse.tile as tile
from concourse import mybir
from concourse.masks import make_identity

FP32 = mybir.dt.float32
BF16 = mybir.dt.bfloat16
INT32 = mybir.dt.int32

# Full-problem constants (per core shard)
S_FULL = 2048
D_FULL = 1024
DML_FULL = 512  # local d_model slice = 8 heads * 64
DK = 64
N_CORES = 8


def build_attention(tc: tile.TileContext, io, S, D, DML):
    """Emit the per-core attention program.

    io: dict of DRAM APs: xq,xk,xv [S,D] f32; mask [S,S] i32; wq,wk,wv [D,DML]
    f32; wo [DML,D] f32; bq,bk,bv [DML] f32; out [S,D] f32.
    """
    nc = tc.nc
    ctx = ExitStack()

    HG = DML // DK          # local heads
    NPAIR = HG // 2
    P = 128
    nS = S // P             # sequence tiles
    nD = D // P             # d_input tiles
    nDM = DML // P          # local d_model tiles (= head pairs)
    SQB = min(512, S)       # sq block (psum free width)
    nSQB = S // SQB
    OCT = min(4, nS)        # s-tiles per transpose group (4 + an 8-deep xe
                            # ring = a full group of DMA prefetch ahead of
                            # the PE transposes; OCT=8 left PE waiting on
                            # loads through setup)
    NOUT = min(512, D)      # out-proj free width
    VST = DK + 2            # V_aug per-head stride (64 data + ones + pad)
    assert nS % OCT == 0 and HG % 2 == 0

    with ctx:
        # ---------------- pools ----------------
        singles = ctx.enter_context(tc.tile_pool(name="singles", bufs=1))
        w_pool = ctx.enter_context(tc.tile_pool(name="w", bufs=1))
        wo_pool = ctx.enter_context(tc.tile_pool(name="wo", bufs=1))
        mstage = ctx.enter_context(tc.tile_pool(name="mstage", bufs=5))
        xe_pool = ctx.enter_context(tc.tile_pool(name="xe", bufs=OCT + 1))
        mo_pool = ctx.enter_context(tc.tile_pool(name="mo", bufs=3))
        xt_pool = ctx.enter_context(tc.tile_pool(name="xt", bufs=1))
        qt_pool = ctx.enter_context(tc.tile_pool(name="qt", bufs=1))
        kt_pool = ctx.enter_context(tc.tile_pool(name="kt", bufs=1))
        va_pool = ctx.enter_context(tc.tile_pool(name="va", bufs=1))
        xo_pool = ctx.enter_context(tc.tile_pool(name="xo", bufs=2))
        mt_pool = ctx.enter_context(tc.tile_pool(name="mt", bufs=min(14, 2 * (nS // 2))))
        p_pool = ctx.enter_context(tc.tile_pool(name="p", bufs=6))
        rd_pool = ctx.enter_context(tc.tile_pool(name="rd", bufs=3))
        out_pool = ctx.enter_context(tc.tile_pool(name="out", bufs=4))
        dram = ctx.enter_context(tc.tile_pool(name="dram", bufs=1, space="DRAM"))

        s_psum = ctx.enter_context(tc.tile_pool(name="s_psum", bufs=2, space="PSUM"))
        pv_psum = ctx.enter_context(tc.tile_pool(name="pv_psum", bufs=2, space="PSUM"))
        m_psum = ctx.enter_context(tc.tile_pool(name="m_psum", bufs=2, space="PSUM"))

        # ---------------- constants ----------------
        identity = singles.tile([P, P], BF16)
        make_identity(nc, identity)
        ones_row = singles.tile([1, P], BF16)
        nc.vector.memset(ones_row, 1.0)
        bq_sb = singles.tile([P, nDM], FP32)
        bk_sb = singles.tile([P, nDM], FP32)
        for b_sb, b_ap in ((bq_sb, io["bq"]), (bk_sb, io["bk"])):
            src = bass.AP(tensor=b_ap.tensor, offset=b_ap.offset, ap=[[1, P], [P, nDM]])
            nc.sync.dma_start(out=b_sb, in_=src)

        # ---------------- mask complement -> DRAM (bf16), per sq block ------
        maskc = [dram.tile([SQB, S], BF16, tag=f"mc{j}", name=f"mc{j}")
                 for j in range(nSQB)]
        MH = 1024 if S % 1024 == 0 else S

        def mask_prep_c(j):
            for r in range(SQB // P):
                row0 = j * SQB + r * P
                for ch in range(S // MH):
                    mi = mstage.tile([P, MH], INT32, tag="mi")
                    # j0 runs in setup where the ACT hwdge queue is idle;
                    # later js stay on SP (ACT is exp-bound in attention)
                    eng = nc.scalar if j == 0 else nc.sync
                    eng.dma_start(out=mi, in_=io["mask"][row0:row0 + P, ch * MH:(ch + 1) * MH])
                    mo = mo_pool.tile([P, MH], BF16, tag="mo")
                    # (mask == 0) -> 1.0 else 0.0
                    nc.vector.tensor_scalar(mo, mi, 0, None, mybir.AluOpType.is_equal)
                    nc.sync.dma_start(out=maskc[j][r * P:(r + 1) * P, ch * MH:(ch + 1) * MH], in_=mo)

        def mask_prep(j):
            mts = [mt_pool.tile([P, 2 * SQB], BF16, tag="mt", name=f"mt{j}_{bt}")
                   for bt in range(nS // 2)]
            for bt in range(nS // 2):
                for half in (0, 1):
                    k = 2 * bt + half
                    nc.sync.dma_start(
                        out=mts[bt][:, half * SQB:(half + 1) * SQB],
                        in_=maskc[j][:, k * P:(k + 1) * P],
                        transpose=True)
            return mts

        # ---------------- x^T + projections ----------------
        def build_xt(x_dram):
            """Load x [S,D] f32, transpose on PE, return nD bf16 tiles [128, S].

            One SWDGE cast DMA per a-group: [128, OCT*D] bf16, partition p
            holding rows a*OCT*P + ii*P + p for ii in range(OCT)."""
            xt = [xt_pool.tile([P, S], BF16, tag=f"xt{dj}", name=f"xt{dj}") for dj in range(nD)]
            for a in range(nS // OCT):
                xe = []
                for ih in range(OCT // 2):
                    # two s-row blocks per SWDGE cast DMA: halves the ~1us
                    # per-DMA issue cost on the Pool engine
                    t = xe_pool.tile([P, 2 * D], BF16, tag="xe2")
                    r0 = (a * OCT + 2 * ih) * P
                    src_ap = x_dram[r0:r0 + 2 * P, :].rearrange("(ii p) d -> p ii d", p=P)
                    nc.gpsimd.dma_start(
                        out=t.rearrange("p (ii d) -> p ii d", ii=2), in_=src_ap)
                    xe.append(t)
                for dj in range(nD):
                    tp = s_psum.tile([P, OCT * P], BF16, tag="s")
                    for ii in range(OCT):
                        nc.tensor.transpose(tp[:, ii * P:(ii + 1) * P],
                                            xe[ii // 2][:, (ii % 2) * D + dj * P:(ii % 2) * D + (dj + 1) * P],
                                            identity)
                    if dj % 4 != 3:
                        nc.vector.tensor_copy(out=xt[dj][:, a * OCT * P:(a + 1) * OCT * P], in_=tp)
                    else:
                        nc.scalar.copy(xt[dj][:, a * OCT * P:(a + 1) * OCT * P], tp)
            return xt

        def load_w(w_dram):
            w = []
            for kt in range(nD):
                ws = mstage.tile([P, DML], FP32, tag="mi")
                nc.sync.dma_start(out=ws, in_=w_dram[kt * P:(kt + 1) * P, :])
                t = w_pool.tile([P, DML], BF16, tag=f"w{kt}")
                nc.vector.tensor_copy(t, ws)
                w.append(t)
            return w

        # Q^T, K^T: [dm, s] tiles
        qt = [qt_pool.tile([P, S], BF16, tag=f"qt{m}", name=f"qtt{m}") for m in range(nDM)]
        kt_t = [kt_pool.tile([P, S], BF16, tag=f"kt{m}", name=f"ktt{m}") for m in range(nDM)]
        for x_dram, w_dram, b_sb, dst in (
            (io["xq"], io["wq"], bq_sb, qt),
        ):
            xt = build_xt(x_dram)
            w = load_w(w_dram)
            for mj in range(nDM):
                for nb in range(nSQB):
                    ps = m_psum.tile([P, SQB], FP32, tag="m")
                    for kj in range(nD):
                        nc.tensor.matmul(ps, w[kj][:, mj * P:(mj + 1) * P],
                                         xt[kj][:, nb * SQB:(nb + 1) * SQB],
                                         start=(kj == 0), stop=(kj == nD - 1))
                    nc.scalar.activation(dst[mj][:, nb * SQB:(nb + 1) * SQB], ps,
                                         mybir.ActivationFunctionType.Identity,
                                         bias=b_sb[:, mj:mj + 1], scale=1.0)

        # j=0 mask chain: no data deps, so it runs during the Q phase whose
        # DMA window is light, keeping the V window free for the xk loads
        mask_prep_c(0)
        mts_all = {0: mask_prep(0)}

        # V natural [s, dm] + ones column per head, bias added via rank-1 matmul
        va = [va_pool.tile([P, HG * VST], BF16, tag=f"va{si}", name=f"vat{si}") for si in range(nS)]
        xt = build_xt(io["xv"])
        w = load_w(io["wv"])
        bv_sb = singles.tile([1, DML], BF16)
        nc.gpsimd.dma_start(out=bv_sb, in_=io["bv"][None, :])
        bv_bc = None  # unused
        for si in range(nS):
            ps = m_psum.tile([P, DML], FP32, tag="m")
            for kj in range(nD):
                nc.tensor.matmul(ps, xt[kj][:, si * P:(si + 1) * P], w[kj],
                                 start=(kj == 0), stop=False)
            nc.tensor.matmul(ps, ones_row, bv_sb, start=False, stop=True)
            va3 = va[si].rearrange("p (h e) -> p h e", e=VST)
            ps3 = ps.rearrange("p (h e) -> p h e", e=DK)
            nc.scalar.copy(va3[:, :, 0:DK], ps3)
            nc.vector.memset(va3[:, :, DK:DK + 1], 1.0)

        # K built LAST: nothing reuses the x^T tiles afterwards, so pairs
        # 1-3 of the K projection can ride inside attention j0 (pair pr only
        # needs kt_t[pr] at its own turn), trimming ~20us off the setup head
        xtk = build_xt(io["xk"])
        wk = load_w(io["wk"])

        def kproj_group(mj, nb, on_pool=False):
            ps = m_psum.tile([P, SQB], FP32, tag="m")
            for kj in range(nD):
                nc.tensor.matmul(ps, wk[kj][:, mj * P:(mj + 1) * P],
                                 xtk[kj][:, nb * SQB:(nb + 1) * SQB],
                                 start=(kj == 0), stop=(kj == nD - 1))
            if on_pool:
                # inside attention j0: keep ACT free for exps.  GPSIMD cannot
                # read PSUM (BIR verifier), so this goes to DVE instead.
                nc.vector.tensor_scalar(
                    kt_t[mj][:, nb * SQB:(nb + 1) * SQB], ps,
                    bk_sb[:, mj:mj + 1], None, mybir.AluOpType.add)
            else:
                nc.scalar.activation(kt_t[mj][:, nb * SQB:(nb + 1) * SQB], ps,
                                     mybir.ActivationFunctionType.Identity,
                                     bias=bk_sb[:, mj:mj + 1], scale=1.0)

        def kproj(mj):
            for nb in range(nSQB):
                kproj_group(mj, nb)

        kproj(0)

        # ---------------- attention + output projection ----------------
        wo = []
        for kj in range(nDM):
            ws = mstage.tile([P, D], FP32, tag="mi")
            nc.sync.dma_start(out=ws, in_=io["wo"][kj * P:(kj + 1) * P, :])
            t = wo_pool.tile([P, D], BF16, tag=f"wo{kj}")
            nc.vector.tensor_copy(t, ws)
            wo.append(t)
        wo_all = None
        def outproj_group(oj, xo, st, nb):
            wp = m_psum.tile([P, NOUT], FP32, tag="m")
            for kj in range(nDM):
                nc.tensor.matmul(
                    wp, xo[kj][:, st * P:(st + 1) * P],
                    wo[kj][:, nb * NOUT:(nb + 1) * NOUT],
                    start=(kj == 0), stop=(kj == nDM - 1))
            ob = out_pool.tile([P, NOUT], FP32, tag="ob")
            nc.vector.tensor_copy(ob, wp)
            nc.sync.dma_start(
                out=io["out"][oj * SQB + st * P:oj * SQB + (st + 1) * P,
                              nb * NOUT:(nb + 1) * NOUT],
                in_=ob)

        def outproj_block(oj, xo):
            for st in range(SQB // P):
                for nb in range(D // NOUT):
                    outproj_group(oj, xo, st, nb)

        from collections import deque
        xo_prev = None
        pend = None
        for j in range(nSQB):
            mts = mts_all.pop(j)
            xo = [xo_pool.tile([P, SQB], BF16, tag=f"xo{m}", name=f"xot{m}_{j}")
                  for m in range(nDM)]
            if j + 1 < nSQB:
                mask_prep_c(j + 1)
                mts_all[j + 1] = mask_prep(j + 1)

            # per-bt PE filler work: outproj groups of j-1 (or kproj groups
            # for j0), spread through the bt loops so boundary stalls vanish
            filler = deque()
            if j == 0:
                # kproj(pr+1) groups ride inside pr's bt loop (ready by pr+1)
                pass
            elif xo_prev is not None:
                oj, xop = j - 1, xo_prev
                for st in range(SQB // P):
                    for nb in range(D // NOUT):
                        filler.append((outproj_group, (oj, xop, st, nb)))
                xo_prev = None

            for pr in range(NPAIR):
                if j == 0 and pr == 0:
                    # kproj(0) nb=1..3 stream just-ahead of their first use
                    # (sc at bt needs key block bt//2): popped at bt 0/2/4
                    filler = deque((kproj_group, (0, nb, True)) for nb in (1, 2, 3))
                if j == 0 and pr >= 1:
                    filler = deque()
                    for _nb in range(nSQB):
                        kproj_group(pr, _nb, on_pool=True)
                pv = [pv_psum.tile([P, SQB], FP32, tag="pv", name=f"pv{j}_{pr}_{_}") for _ in range(2)]

                def emit_pv(bt, pp):
                    for half in (0, 1):
                        k = 2 * bt + half
                        for hh in (0, 1):
                            h = 2 * pr + hh
                            nc.tensor.matmul(
                                pv[hh][0:DK + 1, :],
                                va[k][:, h * VST:h * VST + DK + 1],
                                pp[hh][:, half * SQB:(half + 1) * SQB],
                                start=(k == 0), stop=(k == nS - 1))

                if pend is not None:
                    pend()
                    pend = None
                # filler pacing: kproj groups (1.7us) every 2nd bt; outproj
                # groups (0.85us) every 4th slot across the whole j
                prev = None
                for bt in range(nS // 2):
                    sc = [s_psum.tile([P, 2 * SQB], FP32, tag="s", name=f"sc{j}_{pr}_{bt}_{_}") for _ in range(2)]
                    for half in (0, 1):
                        k = 2 * bt + half
                        for hh in (0, 1):
                            nc.tensor.matmul(
                                sc[hh][:, half * SQB:(half + 1) * SQB],
                                kt_t[pr][hh * DK:(hh + 1) * DK, k * P:(k + 1) * P],
                                qt[pr][hh * DK:(hh + 1) * DK, j * SQB:(j + 1) * SQB],
                                start=True, stop=True)
                    pp = []
                    for hh in (0, 1):
                        t = p_pool.tile([P, 2 * SQB], BF16, tag="p", name=f"pp{j}_{pr}_{bt}_{hh}")
                        nc.scalar.activation(t, sc[hh], mybir.ActivationFunctionType.Exp,
                                             scale=1.0 / np.sqrt(DK))
                        nc.vector.tensor_mul(t, t, mts[bt])
                        pp.append(t)
                    # software pipeline: PV for the previous step rides behind
                    # this step's scores in the PE stream, so the next exp
                    # never waits on PV
                    if prev is not None:
                        emit_pv(*prev)
                    prev = (bt, pp)
                    pace = (bt % 2 == 0) if (j == 0 and pr == 0) else \
                        ((pr * (nS // 2) + bt) % 4 == 0)
                    if filler and pace:
                        fn, fargs = filler.popleft()
                        fn(*fargs)
                emit_pv(*prev)

                def make_norm(pr, pv, xo):
                    def norm():
                        # stage-major across hh so DVE/PE/DVE phases pipeline
                        rdens, rps, rdbs = [], [], []
                        for hh in (0, 1):
                            rden = rd_pool.tile([1, SQB], BF16, tag="rden")
                            with nc.allow_low_precision(reason="softmax rdenom bcast in bf16"):
                                nc.vector.reciprocal(rden, pv[hh][DK:DK + 1, :])
                            rdens.append(rden)
                        for hh in (0, 1):
                            # broadcast partition 0 -> DK partitions, rank-1 matmul
                            rp = m_psum.tile([P, SQB], FP32, tag="m")
                            nc.tensor.matmul(rp[0:DK, :], ones_row[:, 0:DK], rdens[hh],
                                             start=True, stop=True)
                            rps.append(rp)
                        for hh in (0, 1):
                            rdb = rd_pool.tile([DK, SQB], BF16, tag="rdb")
                            nc.vector.tensor_copy(rdb, rps[hh][0:DK, :])
                            rdbs.append(rdb)
                        for hh in (0, 1):
                            nc.vector.scalar_tensor_tensor(
                                out=xo[pr][hh * DK:(hh + 1) * DK, :],
                                in0=pv[hh][0:DK, :], scalar=1.0, in1=rdbs[hh],
                                op0=mybir.AluOpType.bypass, op1=mybir.AluOpType.mult)
                    return norm

                pend = make_norm(pr, pv, xo)
            # leftover filler (if pacing didn't consume it all) drains here
            while filler:
                fn, fargs = filler.popleft()
                fn(*fargs)

            # output projection for this sq block is deferred into j+1;
            # the last pair's norm is deferred into j+1's first bt loop
            xo_prev = xo
        pend()
        outproj_block(nSQB - 1, xo_prev)


def split_excess_waits(nc, default_limit=1, drain_limit=1, dma_limit=1):
    """The walrus build here rejects instructions with too many sem waits
    (Drain/CTRL takes 1).  Hoist excess waits onto same-engine NoOp carriers
    inserted immediately before the offender — semantically identical."""
    n_new = 0
    for f in nc.m.functions:
        for blk in f.blocks:
            insts = blk.instructions
            pos = 0
            while pos < len(insts):
                i = insts[pos]
                if isinstance(i, mybir.InstDrain):
                    limit = drain_limit
                elif isinstance(i, (mybir.InstDMACopy, mybir.InstDmaTransposeAnt)):
                    limit = dma_limit
                else:
                    limit = default_limit
                si = getattr(i, "sync_info", None)
                if si is not None and si.on_wait is not None and len(si.on_wait) > limit:
                    excess = []
                    while len(si.on_wait) > limit:
                        excess.append(si.on_wait.pop())
                    carriers = []
                    for j in range(0, len(excess), max(default_limit, 1)):
                        nd = mybir.InstNoOp(name=f"I-sw{n_new}", ins=[], outs=[])
                        n_new += 1
                        nd.engine = i.engine
                        nd.sync_info = mybir.SyncInfo(
                            on_wait=excess[j:j + default_limit], on_update=[])
                        carriers.append(nd)
                    for k, nd in enumerate(carriers):
                        insts.insert(pos + k, nd)
                    pos += len(carriers)
                pos += 1
    return n_new


def build_nc(S=S_FULL, D=D_FULL, DML=DML_FULL, reps=1, timing=False):
    nc = bass.Bass("TRN2", target_bir_lowering=False, debug=False, num_devices=N_CORES)
    kin = "Internal" if timing else "ExternalInput"
    kout = "Internal" if timing else "ExternalOutput"
    io = {
        "xq": nc.dram_tensor("xq", [S, D], FP32, kind=kin)[:],
        "xk": nc.dram_tensor("xk", [S, D], FP32, kind=kin)[:],
        "xv": nc.dram_tensor("xv", [S, D], FP32, kind=kin)[:],
        "mask": nc.dram_tensor("mask", [S, S], INT32, kind=kin)[:],
        "wq": nc.dram_tensor("wq", [D, DML], FP32, kind=kin)[:],
        "wk": nc.dram_tensor("wk", [D, DML], FP32, kind=kin)[:],
        "wv": nc.dram_tensor("wv", [D, DML], FP32, kind=kin)[:],
        "wo": nc.dram_tensor("wo", [DML, D], FP32, kind=kin)[:],
        "bq": nc.dram_tensor("bq", [DML], FP32, kind=kin)[:],
        "bk": nc.dram_tensor("bk", [DML], FP32, kind=kin)[:],
        "bv": nc.dram_tensor("bv", [DML], FP32, kind=kin)[:],
        "out": nc.dram_tensor("out", [S, D], FP32, kind=kout)[:],
    }
    sink = None
    if timing:
        sink = nc.dram_tensor("sink", [1, 64], FP32, kind="ExternalOutput")[:]
    with tile.TileContext(nc) as tc:
        for _ in range(reps):
            build_attention(tc, io, S, D, DML)
        if sink is not None:
            nc.sync.dma_start(out=sink, in_=io["out"][0:1, 0:64])
    split_excess_waits(nc)
    return nc


_NC_CACHE = {}


def make_in_maps(inputs):
    query = np.asarray(inputs["query"], np.float32)
    key = np.asarray(inputs["key"], np.float32)
    value = np.asarray(inputs["value"], np.float32)
    mask = np.asarray(inputs["mask"], np.int32)
    Wq, bq = np.asarray(inputs["Wq"], np.float32), np.asarray(inputs["bq"], np.float32)
    Wk, bk = np.asarray(inputs["Wk"], np.float32), np.asarray(inputs["bk"], np.float32)
    Wv, bv = np.asarray(inputs["Wv"], np.float32), np.asarray(inputs["bv"], np.float32)
    Wo = np.asarray(inputs["Wo"], np.float32)

    DML = Wq.shape[1] // 2  # head-group slice width

    in_maps = []
    for c in range(N_CORES):
        b, g = divmod(c, 2)
        sl = slice(g * DML, (g + 1) * DML)
        in_maps.append({
            "xq": np.ascontiguousarray(query[b]),
            "xk": np.ascontiguousarray(key[b]),
            "xv": np.ascontiguousarray(value[b]),
            "mask": np.ascontiguousarray(mask[b]),
            "wq": np.ascontiguousarray(Wq[:, sl]),
            "wk": np.ascontiguousarray(Wk[:, sl]),
            "wv": np.ascontiguousarray(Wv[:, sl]),
            "wo": np.ascontiguousarray(Wo[sl, :]),
            "bq": np.ascontiguousarray(bq[sl]),
            "bk": np.ascontiguousarray(bk[sl]),
            "bv": np.ascontiguousarray(bv[sl]),
        })
    return in_maps


def kernel(**inputs):
    B = np.asarray(inputs["query"]).shape[0]
    bo = np.asarray(inputs["bo"], np.float32)

    if "nc" not in _NC_CACHE:
        _NC_CACHE["nc"] = build_nc()
    nc = _NC_CACHE["nc"]

    in_maps = make_in_maps(inputs)

    import os

    from concourse.bass_utils import run_bass_kernel_spmd
    trace = os.environ.get("KERNEL_TRACE", "0") == "1"
    res = run_bass_kernel_spmd(nc, in_maps, core_ids=list(range(N_CORES)), trace=trace)
    _NC_CACHE["last_result"] = res
    out = np.stack([
        res.results[2 * b]["out"] + res.results[2 * b + 1]["out"] + bo
        for b in range(B)
    ]).astype(np.float32)
    return out



# revision 125
# speedup vs baseline: 1.0035x; 1.0035x over previous
"""Multi-head attention (B=4, S=2048, D=1024, 16 heads) on 8 TRN2 NeuronCores.

Sharding: data-parallel over batch (4) x tensor-parallel over heads (2 groups
of 8).  Core c handles batch c//2, head-group c%2.  Each core computes its
head-group's attention and the partial output projection through its slice of
Wo; the host sums the two partials per batch and adds bo.

Per-core kernel (all matmuls bf16, fp32 accumulation):
  - x^T built on-device via PE transposes (inputs cast fp32->bf16 by the
    SWDGE DMA on the otherwise-idle GPSIMD engine; two s-row blocks per
    cast DMA to halve the ~1us SWDGE issue cost that paces the loads)
  - Q^T,K^T = (x Wq/Wk)^T via transposed projections; V natural, augmented
    with a ones column per head (yields softmax denominators for free)
  - scores^T = K_h Q_h^T per head, two heads row-packed in the 128-wide PE
  - softmax without max subtraction (scores ~ N(0,1), no overflow): exp on
    ACT with the 1/sqrt(dk) scale folded in, mask applied as a bf16
    multiply-by-complement on DVE; the complement (mask==0) is produced per
    sq-block just in time, bounced through DRAM, and read back with the DMA
    xbar transpose so mask DMA streams during attention.  The j=0 chain
    issues behind the wv loads so its serialized DMA round trips overlap
    V's projection instead of gating the first exp
  - out_h^T = V_aug^T P^T accumulated on PE; row 64 = denominator.  PV
    matmuls for step bt are emitted AFTER the scores matmuls for step bt+1
    (software pipelining) so the next exp's PSUM input is ready the moment
    ACT finishes the previous exp
  - normalize: DVE reciprocal + rank-1 PE broadcast matmul + DVE
    scalar_tensor_tensor (fused PSUM read, scale, bf16 store)
  - partial out = X Wo_slice in natural layout, DMA'd out in fp32; each
    sq-block's output projection is split into per-(st,nb) groups and
    interleaved one group per 4th bt slot of the NEXT sq block, so the
    j-boundary never clusters 32 matmuls in front of the next exps
  - K projection: only pair 0 / key-block 0 is built ahead of attention;
    pair 0's remaining key blocks stream just-ahead of their first use
    inside pr0's bt loop, and pairs 1-3 are emitted at their pr's turn in
    j0.  The attention-phase psum->SBUF results go through DVE
    (tensor_scalar with per-partition bias) because ACT is exp-saturated
    there and GPSIMD may not read PSUM (BIR verifier rejects it; the
    TimelineSim cost model does not know that constraint — always compile
    before trusting a sim win)

The walrus build here accepts at most one semaphore wait per instruction;
split_excess_waits() hoists extra waits onto NoOp carriers post-scheduling.
"""

import sys

if "/opt/trn_rl_repo" not in sys.path:
    sys.path.insert(0, "/opt/trn_rl_repo")

from contextlib import ExitStack

import numpy as np

import concourse.bass as bass
import concourse.tile as tile
from concourse import mybir
from concourse.masks import make_identity

FP32 = mybir.dt.float32
BF16 = mybir.dt.bfloat16
INT32 = mybir.dt.int32

# Full-problem constants (per core shard)
S_FULL = 2048
D_FULL = 1024
DML_FULL = 512  # local d_model slice = 8 heads * 64
DK = 64
N_CORES = 8


def build_attention(tc: tile.TileContext, io, S, D, DML):
    """Emit the per-core attention program.

    io: dict of DRAM APs: xq,xk,xv [S,D] f32; mask [S,S] i32; wq,wk,wv [D,DML]
    f32; wo [DML,D] f32; bq,bk,bv [DML] f32; out [S,D] f32.
    """
    nc = tc.nc
    ctx = ExitStack()

    HG = DML // DK          # local heads
    NPAIR = HG // 2
    P = 128
    nS = S // P             # sequence tiles
    nD = D // P             # d_input tiles
    nDM = DML // P          # local d_model tiles (= head pairs)
    SQB = min(512, S)       # sq block (psum free width)
    nSQB = S // SQB
    OCT = min(4, nS)        # s-tiles per transpose group (4 + an 8-deep xe
                            # ring = a full group of DMA prefetch ahead of
                            # the PE transposes; OCT=8 left PE waiting on
                            # loads through setup)
    NOUT = min(512, D)      # out-proj free width
    VST = DK + 2            # V_aug per-head stride (64 data + ones + pad)
    assert nS % OCT == 0 and HG % 2 == 0

    with ctx:
        # ---------------- pools ----------------
        singles = ctx.enter_context(tc.tile_pool(name="singles", bufs=1))
        w_pool = ctx.enter_context(tc.tile_pool(name="w", bufs=1))
        wo_pool = ctx.enter_context(tc.tile_pool(name="wo", bufs=1))
        mstage = ctx.enter_context(tc.tile_pool(name="mstage", bufs=5))
        xe_pool = ctx.enter_context(tc.tile_pool(name="xe", bufs=OCT))
        mo_pool = ctx.enter_context(tc.tile_pool(name="mo", bufs=4))
        xt_pool = ctx.enter_context(tc.tile_pool(name="xt", bufs=1))
        qt_pool = ctx.enter_context(tc.tile_pool(name="qt", bufs=1))
        kt_pool = ctx.enter_context(tc.tile_pool(name="kt", bufs=1))
        va_pool = ctx.enter_context(tc.tile_pool(name="va", bufs=1))
        xo_pool = ctx.enter_context(tc.tile_pool(name="xo", bufs=2))
        mt_pool = ctx.enter_context(tc.tile_pool(name="mt", bufs=min(14, 2 * (nS // 2))))
        p_pool = ctx.enter_context(tc.tile_pool(name="p", bufs=6))
        rd_pool = ctx.enter_context(tc.tile_pool(name="rd", bufs=4))
        out_pool = ctx.enter_context(tc.tile_pool(name="out", bufs=4))
        dram = ctx.enter_context(tc.tile_pool(name="dram", bufs=1, space="DRAM"))

        s_psum = ctx.enter_context(tc.tile_pool(name="s_psum", bufs=2, space="PSUM"))
        pv_psum = ctx.enter_context(tc.tile_pool(name="pv_psum", bufs=2, space="PSUM"))
        m_psum = ctx.enter_context(tc.tile_pool(name="m_psum", bufs=2, space="PSUM"))

        # ---------------- constants ----------------
        identity = singles.tile([P, P], BF16)
        make_identity(nc, identity)
        ones_row = singles.tile([1, P], BF16)
        nc.vector.memset(ones_row, 1.0)
        bq_sb = singles.tile([P, nDM], FP32)
        bk_sb = singles.tile([P, nDM], FP32)
        for b_sb, b_ap in ((bq_sb, io["bq"]), (bk_sb, io["bk"])):
            src = bass.AP(tensor=b_ap.tensor, offset=b_ap.offset, ap=[[1, P], [P, nDM]])
            nc.sync.dma_start(out=b_sb, in_=src)

        # ---------------- mask complement -> DRAM (bf16), per sq block ------
        maskc = [dram.tile([SQB, S], BF16, tag=f"mc{j}", name=f"mc{j}")
                 for j in range(nSQB)]
        MH = 1024 if S % 1024 == 0 else S

        def mask_prep_c(j):
            for r in range(SQB // P):
                row0 = j * SQB + r * P
                for ch in range(S // MH):
                    mi = mstage.tile([P, MH], INT32, tag="mi")
                    # j0 runs in setup where the ACT hwdge queue is idle;
                    # later js stay on SP (ACT is exp-bound in attention)
                    eng = nc.scalar if j == 0 else nc.sync
                    eng.dma_start(out=mi, in_=io["mask"][row0:row0 + P, ch * MH:(ch + 1) * MH])
                    mo = mo_pool.tile([P, MH], BF16, tag="mo")
                    # (mask == 0) -> 1.0 else 0.0
                    nc.vector.tensor_scalar(mo, mi, 0, None, mybir.AluOpType.is_equal)
                    nc.sync.dma_start(out=maskc[j][r * P:(r + 1) * P, ch * MH:(ch + 1) * MH], in_=mo)

        def mask_prep(j):
            mts = [mt_pool.tile([P, 2 * SQB], BF16, tag="mt", name=f"mt{j}_{bt}")
                   for bt in range(nS // 2)]
            for bt in range(nS // 2):
                for half in (0, 1):
                    k = 2 * bt + half
                    nc.sync.dma_start(
                        out=mts[bt][:, half * SQB:(half + 1) * SQB],
                        in_=maskc[j][:, k * P:(k + 1) * P],
                        transpose=True)
            return mts

        # ---------------- x^T + projections ----------------
        def build_xt(x_dram):
            """Load x [S,D] f32, transpose on PE, return nD bf16 tiles [128, S].

            One SWDGE cast DMA per a-group: [128, OCT*D] bf16, partition p
            holding rows a*OCT*P + ii*P + p for ii in range(OCT)."""
            xt = [xt_pool.tile([P, S], BF16, tag=f"xt{dj}", name=f"xt{dj}") for dj in range(nD)]
            for a in range(nS // OCT):
                xe = []
                for ih in range(OCT // 2):
                    # two s-row blocks per SWDGE cast DMA: halves the ~1us
                    # per-DMA issue cost on the Pool engine
                    t = xe_pool.tile([P, 2 * D], BF16, tag="xe2")
                    r0 = (a * OCT + 2 * ih) * P
                    src_ap = x_dram[r0:r0 + 2 * P, :].rearrange("(ii p) d -> p ii d", p=P)
                    nc.gpsimd.dma_start(
                        out=t.rearrange("p (ii d) -> p ii d", ii=2), in_=src_ap)
                    xe.append(t)
                for dj in range(nD):
                    tp = s_psum.tile([P, OCT * P], BF16, tag="s")
                    for ii in range(OCT):
                        nc.tensor.transpose(tp[:, ii * P:(ii + 1) * P],
                                            xe[ii // 2][:, (ii % 2) * D + dj * P:(ii % 2) * D + (dj + 1) * P],
                                            identity)
                    if dj % 4 != 3:
                        nc.vector.tensor_copy(out=xt[dj][:, a * OCT * P:(a + 1) * OCT * P], in_=tp)
                    else:
                        nc.scalar.copy(xt[dj][:, a * OCT * P:(a + 1) * OCT * P], tp)
            return xt

        def load_w(w_dram):
            w = []
            for kt in range(nD):
                ws = mstage.tile([P, DML], FP32, tag="mi")
                nc.sync.dma_start(out=ws, in_=w_dram[kt * P:(kt + 1) * P, :])
                t = w_pool.tile([P, DML], BF16, tag=f"w{kt}")
                nc.vector.tensor_copy(t, ws)
                w.append(t)
            return w

        # Q^T, K^T: [dm, s] tiles
        qt = [qt_pool.tile([P, S], BF16, tag=f"qt{m}", name=f"qtt{m}") for m in range(nDM)]
        kt_t = [kt_pool.tile([P, S], BF16, tag=f"kt{m}", name=f"ktt{m}") for m in range(nDM)]
        for x_dram, w_dram, b_sb, dst in (
            (io["xq"], io["wq"], bq_sb, qt),
        ):
            xt = build_xt(x_dram)
            w = load_w(w_dram)
            for mj in range(nDM):
                for nb in range(nSQB):
                    ps = m_psum.tile([P, SQB], FP32, tag="m")
                    for kj in range(nD):
                        nc.tensor.matmul(ps, w[kj][:, mj * P:(mj + 1) * P],
                                         xt[kj][:, nb * SQB:(nb + 1) * SQB],
                                         start=(kj == 0), stop=(kj == nD - 1))
                    nc.scalar.activation(dst[mj][:, nb * SQB:(nb + 1) * SQB], ps,
                                         mybir.ActivationFunctionType.Identity,
                                         bias=b_sb[:, mj:mj + 1], scale=1.0)

        # j=0 mask chain: no data deps, so it runs during the Q phase whose
        # DMA window is light, keeping the V window free for the xk loads
        mask_prep_c(0)
        mts_all = {0: mask_prep(0)}

        # V natural [s, dm] + ones column per head, bias added via rank-1 matmul
        va = [va_pool.tile([P, HG * VST], BF16, tag=f"va{si}", name=f"vat{si}") for si in range(nS)]
        xt = build_xt(io["xv"])
        w = load_w(io["wv"])
        bv_sb = singles.tile([1, DML], BF16)
        nc.gpsimd.dma_start(out=bv_sb, in_=io["bv"][None, :])
        bv_bc = None  # unused
        for si in range(nS):
            ps = m_psum.tile([P, DML], FP32, tag="m")
            for kj in range(nD):
                nc.tensor.matmul(ps, xt[kj][:, si * P:(si + 1) * P], w[kj],
                                 start=(kj == 0), stop=False)
            nc.tensor.matmul(ps, ones_row, bv_sb, start=False, stop=True)
            va3 = va[si].rearrange("p (h e) -> p h e", e=VST)
            ps3 = ps.rearrange("p (h e) -> p h e", e=DK)
            nc.scalar.copy(va3[:, :, 0:DK], ps3)
            nc.vector.memset(va3[:, :, DK:DK + 1], 1.0)

        # K built LAST: nothing reuses the x^T tiles afterwards, so pairs
        # 1-3 of the K projection can ride inside attention j0 (pair pr only
        # needs kt_t[pr] at its own turn), trimming ~20us off the setup head
        xtk = build_xt(io["xk"])
        wk = load_w(io["wk"])

        def kproj_group(mj, nb, on_pool=False):
            ps = m_psum.tile([P, SQB], FP32, tag="m")
            for kj in range(nD):
                nc.tensor.matmul(ps, wk[kj][:, mj * P:(mj + 1) * P],
                                 xtk[kj][:, nb * SQB:(nb + 1) * SQB],
                                 start=(kj == 0), stop=(kj == nD - 1))
            if on_pool:
                # inside attention j0: keep ACT free for exps.  GPSIMD cannot
                # read PSUM (BIR verifier), so this goes to DVE instead.
                nc.vector.tensor_scalar(
                    kt_t[mj][:, nb * SQB:(nb + 1) * SQB], ps,
                    bk_sb[:, mj:mj + 1], None, mybir.AluOpType.add)
            else:
                nc.scalar.activation(kt_t[mj][:, nb * SQB:(nb + 1) * SQB], ps,
                                     mybir.ActivationFunctionType.Identity,
                                     bias=bk_sb[:, mj:mj + 1], scale=1.0)

        def kproj(mj):
            for nb in range(nSQB):
                kproj_group(mj, nb)

        kproj(0)

        # ---------------- attention + output projection ----------------
        wo = []
        for kj in range(nDM):
            ws = mstage.tile([P, D], FP32, tag="mi")
            nc.sync.dma_start(out=ws, in_=io["wo"][kj * P:(kj + 1) * P, :])
            t = wo_pool.tile([P, D], BF16, tag=f"wo{kj}")
            nc.vector.tensor_copy(t, ws)
            wo.append(t)
        wo_all = None
        def outproj_group(oj, xo, st, nb):
            wp = m_psum.tile([P, NOUT], FP32, tag="m")
            for kj in range(nDM):
                nc.tensor.matmul(
                    wp, xo[kj][:, st * P:(st + 1) * P],
                    wo[kj][:, nb * NOUT:(nb + 1) * NOUT],
                    start=(kj == 0), stop=(kj == nDM - 1))
            ob = out_pool.tile([P, NOUT], FP32, tag="ob")
            nc.vector.tensor_copy(ob, wp)
            nc.sync.dma_start(
                out=io["out"][oj * SQB + st * P:oj * SQB + (st + 1) * P,
                              nb * NOUT:(nb + 1) * NOUT],
                in_=ob)

        def outproj_block(oj, xo):
            for st in range(SQB // P):
                for nb in range(D // NOUT):
                    outproj_group(oj, xo, st, nb)

        from collections import deque
        xo_prev = None
        pend = None
        for j in range(nSQB):
            mts = mts_all.pop(j)
            xo = [xo_pool.tile([P, SQB], BF16, tag=f"xo{m}", name=f"xot{m}_{j}")
                  for m in range(nDM)]
            if j + 1 < nSQB:
                mask_prep_c(j + 1)
                mts_all[j + 1] = mask_prep(j + 1)

            # per-bt PE filler work: outproj groups of j-1 (or kproj groups
            # for j0), spread through the bt loops so boundary stalls vanish
            filler = deque()
            if j == 0:
                # kproj(pr+1) groups ride inside pr's bt loop (ready by pr+1)
                pass
            elif xo_prev is not None:
                oj, xop = j - 1, xo_prev
                for st in range(SQB // P):
                    for nb in range(D // NOUT):
                        filler.append((outproj_group, (oj, xop, st, nb)))
                xo_prev = None

            for pr in range(NPAIR):
                if j == 0 and pr == 0:
                    # kproj(0) nb=1..3 stream just-ahead of their first use
                    # (sc at bt needs key block bt//2): popped at bt 0/2/4
                    filler = deque((kproj_group, (0, nb, True)) for nb in (1, 2, 3))
                if j == 0 and pr >= 1:
                    filler = deque()
                    for _nb in range(nSQB):
                        kproj_group(pr, _nb, on_pool=True)
                pv = [pv_psum.tile([P, SQB], FP32, tag="pv", name=f"pv{j}_{pr}_{_}") for _ in range(2)]

                def emit_pv(bt, pp):
                    for half in (0, 1):
                        k = 2 * bt + half
                        for hh in (0, 1):
                            h = 2 * pr + hh
                            nc.tensor.matmul(
                                pv[hh][0:DK + 1, :],
                                va[k][:, h * VST:h * VST + DK + 1],
                                pp[hh][:, half * SQB:(half + 1) * SQB],
                                start=(k == 0), stop=(k == nS - 1))

                if pend is not None:
                    pend()
                    pend = None
                # filler pacing: kproj groups (1.7us) every 2nd bt; outproj
                # groups (0.85us) every 4th slot across the whole j
                prev = None
                for bt in range(nS // 2):
                    sc = [s_psum.tile([P, 2 * SQB], FP32, tag="s", name=f"sc{j}_{pr}_{bt}_{_}") for _ in range(2)]
                    for half in (0, 1):
                        k = 2 * bt + half
                        for hh in (0, 1):
                            nc.tensor.matmul(
                                sc[hh][:, half * SQB:(half + 1) * SQB],
                                kt_t[pr][hh * DK:(hh + 1) * DK, k * P:(k + 1) * P],
                                qt[pr][hh * DK:(hh + 1) * DK, j * SQB:(j + 1) * SQB],
                                start=True, stop=True)
                    pp = []
                    for hh in (0, 1):
                        t = p_pool.tile([P, 2 * SQB], BF16, tag="p", name=f"pp{j}_{pr}_{bt}_{hh}")
                        nc.scalar.activation(t, sc[hh], mybir.ActivationFunctionType.Exp,
                                             scale=1.0 / np.sqrt(DK))
                        nc.vector.tensor_mul(t, t, mts[bt])
                        pp.append(t)
                    # software pipeline: PV for the previous step rides behind
                    # this step's scores in the PE stream, so the next exp
                    # never waits on PV
                    if prev is not None:
                        emit_pv(*prev)
                    prev = (bt, pp)
                    pace = (bt % 2 == 0) if (j == 0 and pr == 0) else \
                        ((pr * (nS // 2) + bt) % 4 == 0)
                    if filler and pace:
                        fn, fargs = filler.popleft()
                        fn(*fargs)
                emit_pv(*prev)

                def make_norm(pr, pv, xo):
                    def norm():
                        # stage-major across hh so DVE/PE/DVE phases pipeline
                        rdens, rps, rdbs = [], [], []
                        for hh in (0, 1):
                            rden = rd_pool.tile([1, SQB], BF16, tag="rden")
                            with nc.allow_low_precision(reason="softmax rdenom bcast in bf16"):
                                nc.vector.reciprocal(rden, pv[hh][DK:DK + 1, :])
                            rdens.append(rden)
                        for hh in (0, 1):
                            # broadcast partition 0 -> DK partitions, rank-1 matmul
                            rp = m_psum.tile([P, SQB], FP32, tag="m")
                            nc.tensor.matmul(rp[0:DK, :], ones_row[:, 0:DK], rdens[hh],
                                             start=True, stop=True)
                            rps.append(rp)
                        for hh in (0, 1):
                            rdb = rd_pool.tile([DK, SQB], BF16, tag="rdb")
                            nc.vector.tensor_copy(rdb, rps[hh][0:DK, :])
                            rdbs.append(rdb)
                        for hh in (0, 1):
                            nc.vector.scalar_tensor_tensor(
                                out=xo[pr][hh * DK:(hh + 1) * DK, :],
                                in0=pv[hh][0:DK, :], scalar=1.0, in1=rdbs[hh],
                                op0=mybir.AluOpType.bypass, op1=mybir.AluOpType.mult)
                    return norm

                pend = make_norm(pr, pv, xo)
            # leftover filler (if pacing didn't consume it all) drains here
            while filler:
                fn, fargs = filler.popleft()
                fn(*fargs)

            # output projection for this sq block is deferred into j+1;
            # the last pair's norm is deferred into j+1's first bt loop
            xo_prev = xo
        pend()
        outproj_block(nSQB - 1, xo_prev)


def split_excess_waits(nc, default_limit=1, drain_limit=1, dma_limit=1):
    """The walrus build here rejects instructions with too many sem waits
    (Drain/CTRL takes 1).  Hoist excess waits onto same-engine NoOp carriers
    inserted immediately before the offender — semantically identical."""
    n_new = 0
    for f in nc.m.functions:
        for blk in f.blocks:
            insts = blk.instructions
            pos = 0
            while pos < len(insts):
                i = insts[pos]
                if isinstance(i, mybir.InstDrain):
                    limit = drain_limit
                elif isinstance(i, (mybir.InstDMACopy, mybir.InstDmaTransposeAnt)):
                    limit = dma_limit
                else:
                    limit = default_limit
                si = getattr(i, "sync_info", None)
                if si is not None and si.on_wait is not None and len(si.on_wait) > limit:
                    excess = []
                    while len(si.on_wait) > limit:
                        excess.append(si.on_wait.pop())
                    carriers = []
                    for j in range(0, len(excess), max(default_limit, 1)):
                        nd = mybir.InstNoOp(name=f"I-sw{n_new}", ins=[], outs=[])
                        n_new += 1
                        nd.engine = i.engine
                        nd.sync_info = mybir.SyncInfo(
                            on_wait=excess[j:j + default_limit], on_update=[])
                        carriers.append(nd)
                    for k, nd in enumerate(carriers):
                        insts.insert(pos + k, nd)
                    pos += len(carriers)
                pos += 1
    return n_new


def build_nc(S=S_FULL, D=D_FULL, DML=DML_FULL, reps=1, timing=False):
    nc = bass.Bass("TRN2", target_bir_lowering=False, debug=False, num_devices=N_CORES)
    kin = "Internal" if timing else "ExternalInput"
    kout = "Internal" if timing else "ExternalOutput"
    io = {
        "xq": nc.dram_tensor("xq", [S, D], FP32, kind=kin)[:],
        "xk": nc.dram_tensor("xk", [S, D], FP32, kind=kin)[:],
        "xv": nc.dram_tensor("xv", [S, D], FP32, kind=kin)[:],
        "mask": nc.dram_tensor("mask", [S, S], INT32, kind=kin)[:],
        "wq": nc.dram_tensor("wq", [D, DML], FP32, kind=kin)[:],
        "wk": nc.dram_tensor("wk", [D, DML], FP32, kind=kin)[:],
        "wv": nc.dram_tensor("wv", [D, DML], FP32, kind=kin)[:],
        "wo": nc.dram_tensor("wo", [DML, D], FP32, kind=kin)[:],
        "bq": nc.dram_tensor("bq", [DML], FP32, kind=kin)[:],
        "bk": nc.dram_tensor("bk", [DML], FP32, kind=kin)[:],
        "bv": nc.dram_tensor("bv", [DML], FP32, kind=kin)[:],
        "out": nc.dram_tensor("out", [S, D], FP32, kind=kout)[:],
    }
    sink = None
    if timing:
        sink = nc.dram_tensor("sink", [1, 64], FP32, kind="ExternalOutput")[:]
    with tile.TileContext(nc) as tc:
        for _ in range(reps):
            build_attention(tc, io, S, D, DML)
        if sink is not None:
            nc.sync.dma_start(out=sink, in_=io["out"][0:1, 0:64])
    split_excess_waits(nc)
    return nc


_NC_CACHE = {}


def make_in_maps(inputs):
    query = np.asarray(inputs["query"], np.float32)
    key = np.asarray(inputs["key"], np.float32)
    value = np.asarray(inputs["value"], np.float32)
    mask = np.asarray(inputs["mask"], np.int32)
    Wq, bq = np.asarray(inputs["Wq"], np.float32), np.asarray(inputs["bq"], np.float32)
    Wk, bk = np.asarray(inputs["Wk"], np.float32), np.asarray(inputs["bk"], np.float32)
    Wv, bv = np.asarray(inputs["Wv"], np.float32), np.asarray(inputs["bv"], np.float32)
    Wo = np.asarray(inputs["Wo"], np.float32)

    DML = Wq.shape[1] // 2  # head-group slice width

    in_maps = []
    for c in range(N_CORES):
        b, g = divmod(c, 2)
        sl = slice(g * DML, (g + 1) * DML)
        in_maps.append({
            "xq": np.ascontiguousarray(query[b]),
            "xk": np.ascontiguousarray(key[b]),
            "xv": np.ascontiguousarray(value[b]),
            "mask": np.ascontiguousarray(mask[b]),
            "wq": np.ascontiguousarray(Wq[:, sl]),
            "wk": np.ascontiguousarray(Wk[:, sl]),
            "wv": np.ascontiguousarray(Wv[:, sl]),
            "wo": np.ascontiguousarray(Wo[sl, :]),
            "bq": np.ascontiguousarray(bq[sl]),
            "bk": np.ascontiguousarray(bk[sl]),
            "bv": np.ascontiguousarray(bv[sl]),
        })
    return in_maps


def kernel(**inputs):
    B = np.asarray(inputs["query"]).shape[0]
    bo = np.asarray(inputs["bo"], np.float32)

    if "nc" not in _NC_CACHE:
        _NC_CACHE["nc"] = build_nc()
    nc = _NC_CACHE["nc"]

    in_maps = make_in_maps(inputs)

    import os

    from concourse.bass_utils import run_bass_kernel_spmd
    trace = os.environ.get("KERNEL_TRACE", "0") == "1"
    res = run_bass_kernel_spmd(nc, in_maps, core_ids=list(range(N_CORES)), trace=trace)
    _NC_CACHE["last_result"] = res
    out = np.stack([
        res.results[2 * b]["out"] + res.results[2 * b + 1]["out"] + bo
        for b in range(B)
    ]).astype(np.float32)
    return out

